# revision 6
# baseline (speedup 1.0000x reference)
"""LogicLayer Trainium2 kernel: out = c0 + c1*x[:,ia] + c2*x[:,ib] + c3*x[:,ia]*x[:,ib]
with coef = softmax(weights) @ OP_COEFFS, computed on-device.

Sharding: data-parallel over batch (2048/8 = 256 rows per core).
Per core: PE-transpose x-slice -> xT [16384,256] in DRAM scratch, then SWDGE
dma_gather rows of xT (4-queue rotation), ACT scale/bias + DVE tensor-tensor
compute in the transposed domain, PE transpose back, DMA out.
"""
import sys

sys.path.insert(0, "/opt/trn_rl_repo")
import numpy as np

import concourse.bass as bass  # noqa: F401
import concourse.bacc as bacc
from concourse import mybir
from concourse.bass_utils import run_bass_kernel_spmd

_OP_COEFFS = np.array([
    [0., 0., 0., 0.], [0., 0., 0., 1.], [0., 1., 0., -1.], [0., 1., 0., 0.],
    [0., 0., 1., -1.], [0., 0., 1., 0.], [0., 1., 1., -2.], [0., 1., 1., -1.],
    [1., -1., -1., 1.], [1., -1., -1., 2.], [1., 0., -1., 0.], [1., 0., -1., 1.],
    [1., -1., 0., 0.], [1., -1., 0., 1.], [1., 0., 0., -1.], [1., 0., 0., 0.],
], dtype=np.float32)

BATCH, IN_DIM, OUT_DIM = 2048, 16384, 16384
NCORES = 8
B = BATCH // NCORES          # 256 per-core batch rows
NBH = B // 128               # 2 batch halves
NG = 1024                    # out-neurons per gather group
NGROUP = OUT_DIM // NG       # 16 groups
NCH = NG // 128              # 8 chunks per group
NBLK = NBH * 16              # 32 phase-1 blocks of [128, 1024]
F32 = mybir.dt.float32
I16 = mybir.dt.int16
AX = mybir.AxisListType.X
IDENT = mybir.ActivationFunctionType.Identity
EXP = mybir.ActivationFunctionType.Exp

_cached = {}


def build_nc():
    nc = bacc.Bacc("TRN2", target_bir_lowering=False, num_swdge_queues=4)
    xs = nc.declare_dram_parameter("xs", [B, IN_DIM], F32, isOutput=False)
    wc = nc.declare_dram_parameter("wc", [128, 128 * 16], F32, isOutput=False)
    opc_in = nc.declare_dram_parameter("opc", [128, 4 * 2048], F32, isOutput=False)
    ident_in = nc.declare_dram_parameter("ident", [128, 128], F32, isOutput=False)
    ia_in = nc.declare_dram_parameter("ia", [128, OUT_DIM // 16], I16, isOutput=False)
    ib_in = nc.declare_dram_parameter("ib", [128, OUT_DIM // 16], I16, isOutput=False)
    out = nc.declare_dram_parameter("out", [B, OUT_DIM], F32, isOutput=True)
    xT = nc.dram_tensor("xT", [IN_DIM, B], F32)
    xT_r = xT.ap().rearrange("(a p) b -> p a b", p=128)  # [p, a, b] = xT[a*128+p, b]

    from contextlib import ExitStack
    es = ExitStack()
    sb = lambda n, shape, dt=F32: es.enter_context(nc.sbuf_tensor(n, shape, dt))
    ps = lambda n: es.enter_context(nc.psum_tensor(n, [128, 512], F32))
    sem = lambda n: es.enter_context(nc.semaphore(n))
    ident = sb("ident_t", [128, 128]); wt = sb("wt", [128, 2048]); mk = sb("mk", [128, 2048])
    opcf = sb("opcf", [128, 4, 2048]); ssum = sb("ssum", [128, 128]); rinv = sb("rinv", [128, 128])
    ckn = sb("ckn", [128, 128]); ck = sb("ck", [128, 512])
    iat = sb("iat", [128, OUT_DIM // 16], I16); ibt = sb("ibt", [128, OUT_DIM // 16], I16)
    xin = sb("xin", [128, 2, 1024]); stg = sb("stg", [128, 2, 1024])
    ga = sb("ga", [128, 2, NCH, B]); gb = sb("gb", [128, 2, NCH, B])
    uf = sb("uf", [128, NG * 2]); wf = sb("wf", [128, NG * 2])
    ot = sb("ot", [128, 2, NG * 2]); so = sb("so", [128, 2, 2, NG])
    p0 = ps("p0"); p1 = ps("p1"); p2 = ps("p2"); p3 = ps("p3"); pc = ps("pc")
    ldw = sem("ldw"); cfA = sem("cfA"); cfD = sem("cfD"); cfP = sem("cfP")
    cfE = sem("cfE"); ld1 = sem("ld1"); pe1 = sem("pe1"); peb = sem("peb")
    ev1 = sem("ev1"); st1 = sem("st1"); gs = sem("gs"); act2 = sem("act2")
    tt2 = sem("tt2"); pe2 = sem("pe2"); ev2 = sem("ev2"); sos = sem("sos")
    with es, nc.Block() as block:
        psum = [p0, p1, p2, p3]

        @block.sync
        def _(sync):
            sync.dma_start(wt[:], wc[:]).then_inc(ldw, 16)
            sync.dma_start(ident[:], ident_in[:]).then_inc(ldw, 16)
            sync.dma_start(iat[:], ia_in[:]).then_inc(ldw, 16)
            sync.dma_start(ibt[:], ib_in[:]).then_inc(ldw, 16)
            sync.dma_start(opcf.ap().rearrange("p a b -> p (a b)"), opc_in[:]).then_inc(ldw, 16)
            # phase 1: stream x blocks in, transposed stages out
            for i in range(NBLK):
                bh, g = i // 16, i % 16
                if i >= 2:
                    sync.wait_ge(pe1, 2 * (i - 1))
                sync.dma_start(
                    xin[:, i % 2, :], xs[bh * 128:(bh + 1) * 128, g * 1024:(g + 1) * 1024]
                ).then_inc(ld1, 16)
                sync.wait_ge(ev1, 2 * (i + 1))
                sync.dma_start(
                    xT_r[:, g * 8:(g + 1) * 8, bh * 128:(bh + 1) * 128],
                    stg[:, i % 2, :].rearrange("p (a b) -> p a b", a=8),
                ).then_inc(st1, 16)
            # phase 2: output writes
            for g in range(NGROUP):
                sync.wait_ge(ev2, 4 * (g + 1))
                for bh in range(NBH):
                    sync.dma_start(
                        out[bh * 128:(bh + 1) * 128, g * NG:(g + 1) * NG],
                        so[:, g % 2, bh, :],
                    ).then_inc(sos, 16)

        @block.scalar
        def _(act):
            act.wait_ge(ldw, 16)
            act.activation(wt[:], wt[:], EXP).then_inc(cfA, 1)
            act.wait_ge(cfE, 1)
            for g in range(NGROUP):
                act.wait_ge(gs, 32 * g + 32)          # gb ready
                if g >= 1:
                    act.wait_ge(tt2, 2 * g)           # uf/wf free
                for c in range(NCH):
                    r = g * NCH + c
                    act.activation(                    # u = c3*b + c1
                        uf[:, c * B:(c + 1) * B], gb[:, g % 2, c, :], IDENT,
                        bias=ck[:, 128 + r:129 + r], scale=ck[:, 384 + r:385 + r],
                    ).then_inc(act2, 1)
                    act.activation(                    # w = c2*b + c0
                        wf[:, c * B:(c + 1) * B], gb[:, g % 2, c, :], IDENT,
                        bias=ck[:, r:r + 1], scale=ck[:, 256 + r:257 + r],
                    ).then_inc(act2, 1)

        @block.vector
        def _(vec):
            # coef chain
            vec.wait_ge(cfA, 1)
            e3 = wt.ap().rearrange("p (a b) -> p a b", b=16)
            m3 = mk.ap().rearrange("p (a b) -> p a b", b=16)
            vec.reduce_sum(ssum[:], e3, axis=AX)
            vec.reciprocal(rinv[:], ssum[:])
            vec.wait_ge(ldw, 80)
            for k in range(4):
                o3c = opcf[:, k, :].rearrange("p (a b) -> p a b", b=16)
                vec.tensor_mul(m3, e3, o3c)
                vec.reduce_sum(ckn[:] if k == 0 else ckn[:], m3, axis=AX)
                if k >= 1:
                    vec.wait_ge(cfP, k)
                vec.tensor_mul(ckn[:], ckn[:], rinv[:]).then_inc(cfD, 1)
            vec.wait_ge(cfP, 4)
            vec.tensor_copy(ck[:], pc[:]).then_inc(cfE, 1)
            # phase 1 psum evacs
            for i in range(NBLK):
                for h in range(2):
                    u = 2 * i + h
                    vec.wait_ge(pe1, u + 1)
                    if h == 0 and i >= 2:
                        vec.wait_ge(st1, 16 * (i - 1))
                    vec.tensor_copy(
                        stg[:, i % 2, h * 512:(h + 1) * 512], psum[u % 4][:]
                    ).then_inc(ev1, 1)
            # phase 2 compute + evacs
            for g in range(NGROUP):
                vec.wait_ge(act2, 16 * (g + 1))
                vec.wait_ge(gs, 32 * g + 16)          # ga ready
                vec.tensor_mul(
                    mk[:], ga[:, g % 2].rearrange("p c e -> p (c e)"), uf[:]
                ).then_inc(tt2, 1)
                if g >= 2:
                    vec.wait_ge(pe2, 4 * (g - 1))     # ot slot free
                vec.tensor_add(ot[:, g % 2, :], mk[:], wf[:]).then_inc(tt2, 1)
                for s in range(4):
                    v = 4 * g + s
                    vec.wait_ge(pe2, v + 1)
                    if s == 0 and g >= 2:
                        vec.wait_ge(sos, 32 * (g - 1))
                    bh, sp = s % 2, s // 2
                    vec.tensor_copy(
                        so[:, g % 2, bh, sp * 512:(sp + 1) * 512], psum[v % 4][:]
                    ).then_inc(ev2, 1)

        @block.tensor
        def _(pe):
            pe.wait_ge(ldw, 32)
            for k in range(4):
                pe.wait_ge(cfD, k + 1)
                pe.transpose(pc[:, k * 128:(k + 1) * 128], ckn[:], ident[:]).then_inc(cfP, 1)
            # phase 1
            for i in range(NBLK):
                pe.wait_ge(ld1, 16 * (i + 1))
                for t in range(8):
                    u = 2 * i + t // 4
                    if t % 4 == 0 and u >= 4:
                        pe.wait_ge(ev1, u - 3)
                    inst = pe.transpose(
                        psum[u % 4][:, (t % 4) * 128:(t % 4 + 1) * 128],
                        xin[:, i % 2, t * 128:(t + 1) * 128],
                        ident[:],
                    )
                    if t % 4 == 3:
                        inst.then_inc(pe1, 1)
            # phase 2 back-transposes
            for g in range(NGROUP):
                pe.wait_ge(tt2, 2 * g + 2)
                o3 = ot[:, g % 2, :].rearrange("p (c e) -> p c e", c=NCH)
                for c in range(NCH):
                    for bh in range(2):
                        v = 4 * g + 2 * (c // 4) + bh
                        if c % 4 == 0 and v >= 4:
                            pe.wait_ge(ev2, v - 3)
                        inst = pe.transpose(
                            psum[v % 4][:, (c % 4) * 128:(c % 4 + 1) * 128],
                            o3[:, c, bh * 128:(bh + 1) * 128],
                            ident[:],
                        )
                        if c % 4 == 3:
                            inst.then_inc(pe2, 1)

        @block.gpsimd
        def _(gp):
            gp.wait_ge(st1, 16 * NBLK)
            gp.wait_ge(ldw, 64)
            for g in range(NGROUP):
                if g >= 2:
                    gp.wait_ge(tt2, 2 * (g - 2) + 1)
                gp.dma_gather(
                    ga[:, g % 2], xT[:], iat[:, g * (NG // 16):(g + 1) * (NG // 16)],
                    num_idxs=NG, num_idxs_reg=NG, elem_size=B,
                    single_packet=False, queue_num=(2 * g) % 4,
                ).then_inc(gs, 16)
                if g >= 2:
                    gp.wait_ge(act2, 16 * (g - 1))
                gp.dma_gather(
                    gb[:, g % 2], xT[:], ibt[:, g * (NG // 16):(g + 1) * (NG // 16)],
                    num_idxs=NG, num_idxs_reg=NG, elem_size=B,
                    single_packet=False, queue_num=(2 * g + 1) % 4,
                ).then_inc(gs, 16)

    nc.compile()
    return nc


def wrap_idx(vals):
    """Per-NG-call wrapped int16 tables, concatenated: [128, OUT_DIM//16]."""
    cols = []
    for g in range(NGROUP):
        v = np.asarray(vals[g * NG:(g + 1) * NG])
        arr = v.reshape(NG // 16, 16).T.astype(np.int16)   # [16, NG//16]
        cols.append(np.tile(arr, (8, 1)))                  # [128, NG//16]
    return np.ascontiguousarray(np.concatenate(cols, axis=1))


def kernel(x, idx_a, idx_b, weights, trace=False):
    x = np.asarray(x, dtype=np.float32)
    weights = np.asarray(weights, dtype=np.float32)

    if "nc" not in _cached:
        _cached["nc"] = build_nc()
    nc = _cached["nc"]

    ia_w = wrap_idx(np.asarray(idx_a))
    ib_w = wrap_idx(np.asarray(idx_b))
    wc = np.ascontiguousarray(weights.reshape(128, 128 * 16))
    opc_row = np.repeat(_OP_COEFFS.T[:, None, :], 128, axis=1).reshape(4 * 2048)
    opc = np.ascontiguousarray(np.broadcast_to(opc_row[None, :], (128, 4 * 2048))).astype(np.float32)
    ident = np.eye(128, dtype=np.float32)

    in_maps = [{
        "xs": np.ascontiguousarray(x[k * B:(k + 1) * B]),
        "wc": wc, "opc": opc, "ident": ident, "ia": ia_w, "ib": ib_w,
    } for k in range(NCORES)]
    res = run_bass_kernel_spmd(nc, in_maps, core_ids=list(range(NCORES)), trace=trace)
    out = np.concatenate([r["out"] for r in res.results], axis=0)
    kernel.last_exec_time_ns = res.exec_time_ns
    return out


kernel.last_exec_time_ns = None


# revision 9
# speedup vs baseline: 1.0393x; 1.0393x over previous
"""LogicLayer Trainium2 kernel: out = c0 + c1*x[:,ia] + c2*x[:,ib] + c3*x[:,ia]*x[:,ib]
with coef = softmax(weights) @ OP_COEFFS, computed on-device.

Sharding: data-parallel over batch (2048/8 = 256 rows per core).
Per core: PE-transpose x-slice -> xT [16384,256] in DRAM scratch, then SWDGE
dma_gather rows of xT (4-queue rotation), ACT scale/bias + DVE tensor-tensor
compute in the transposed domain, PE transpose back, DMA out.
"""
import sys

sys.path.insert(0, "/opt/trn_rl_repo")
import numpy as np

import concourse.bass as bass  # noqa: F401
import concourse.bacc as bacc
from concourse import mybir
from concourse.bass_utils import run_bass_kernel_spmd

_OP_COEFFS = np.array([
    [0., 0., 0., 0.], [0., 0., 0., 1.], [0., 1., 0., -1.], [0., 1., 0., 0.],
    [0., 0., 1., -1.], [0., 0., 1., 0.], [0., 1., 1., -2.], [0., 1., 1., -1.],
    [1., -1., -1., 1.], [1., -1., -1., 2.], [1., 0., -1., 0.], [1., 0., -1., 1.],
    [1., -1., 0., 0.], [1., -1., 0., 1.], [1., 0., 0., -1.], [1., 0., 0., 0.],
], dtype=np.float32)

BATCH, IN_DIM, OUT_DIM = 2048, 16384, 16384
NCORES = 8
B = BATCH // NCORES          # 256 per-core batch rows
NBH = B // 128               # 2 batch halves
NG = 1024                    # out-neurons per gather group
NGROUP = OUT_DIM // NG       # 16 groups
NCH = NG // 128              # 8 chunks per group
NBLK = NBH * 16              # 32 phase-1 blocks of [128, 1024]
F32 = mybir.dt.float32
I16 = mybir.dt.int16
AX = mybir.AxisListType.X
IDENT = mybir.ActivationFunctionType.Identity
EXP = mybir.ActivationFunctionType.Exp

_cached = {}


def build_nc():
    nc = bacc.Bacc("TRN2", target_bir_lowering=False, num_swdge_queues=4)
    xs = nc.declare_dram_parameter("xs", [B, IN_DIM], F32, isOutput=False)
    wc = nc.declare_dram_parameter("wc", [128, 128 * 16], F32, isOutput=False)
    opc_in = nc.declare_dram_parameter("opc", [128, 4 * 2048], F32, isOutput=False)
    ident_in = nc.declare_dram_parameter("ident", [128, 128], F32, isOutput=False)
    ia_in = nc.declare_dram_parameter("ia", [128, OUT_DIM // 16], I16, isOutput=False)
    ib_in = nc.declare_dram_parameter("ib", [128, OUT_DIM // 16], I16, isOutput=False)
    out = nc.declare_dram_parameter("out", [B, OUT_DIM], F32, isOutput=True)
    xT = nc.dram_tensor("xT", [IN_DIM, B], F32)
    xT_r = xT.ap().rearrange("(a p) b -> p a b", p=128)  # [p, a, b] = xT[a*128+p, b]

    from contextlib import ExitStack
    es = ExitStack()
    sb = lambda n, shape, dt=F32: es.enter_context(nc.sbuf_tensor(n, shape, dt))
    ps = lambda n: es.enter_context(nc.psum_tensor(n, [128, 512], F32))
    sem = lambda n: es.enter_context(nc.semaphore(n))
    ident = sb("ident_t", [128, 128]); wt = sb("wt", [128, 2048]); mk = sb("mk", [128, 2048])
    opcf = sb("opcf", [128, 4, 2048]); ssum = sb("ssum", [128, 128]); rinv = sb("rinv", [128, 128])
    ckn = sb("ckn", [128, 128]); ck = sb("ck", [128, 512])
    iat = sb("iat", [128, OUT_DIM // 16], I16); ibt = sb("ibt", [128, OUT_DIM // 16], I16)
    xin = sb("xin", [128, 2, 1024]); stg = sb("stg", [128, 2, 1024])
    ga = sb("ga", [128, 3, NCH, B]); gb = sb("gb", [128, 3, NCH, B])
    uf = sb("uf", [128, NG * 2]); wf = sb("wf", [128, NG * 2])
    ot = sb("ot", [128, 2, NG * 2]); so = sb("so", [128, 2, 2, NG])
    p0 = ps("p0"); p1 = ps("p1"); p2 = ps("p2"); p3 = ps("p3"); pc = ps("pc")
    ldw = sem("ldw"); cfA = sem("cfA"); cfD = sem("cfD"); cfP = sem("cfP")
    cfE = sem("cfE"); ld1 = sem("ld1"); pe1 = sem("pe1"); peb = sem("peb")
    ev1 = sem("ev1"); st1 = sem("st1"); gs = sem("gs"); act2 = sem("act2")
    tt2 = sem("tt2"); pe2 = sem("pe2"); ev2 = sem("ev2"); sos = sem("sos")
    with es, nc.Block() as block:
        psum = [p0, p1, p2, p3]

        @block.sync
        def _(sync):
            sync.dma_start(wt[:], wc[:]).then_inc(ldw, 16)
            sync.dma_start(ident[:], ident_in[:]).then_inc(ldw, 16)
            sync.dma_start(iat[:], ia_in[:]).then_inc(ldw, 16)
            sync.dma_start(ibt[:], ib_in[:]).then_inc(ldw, 16)
            sync.dma_start(opcf.ap().rearrange("p a b -> p (a b)"), opc_in[:]).then_inc(ldw, 16)
            # phase 1: stream x blocks in, transposed stages out
            for i in range(NBLK):
                bh, g = i // 16, i % 16
                if i >= 2:
                    sync.wait_ge(pe1, 2 * (i - 1))
                sync.dma_start(
                    xin[:, i % 2, :], xs[bh * 128:(bh + 1) * 128, g * 1024:(g + 1) * 1024]
                ).then_inc(ld1, 16)
                sync.wait_ge(ev1, 2 * (i + 1))
                sync.dma_start(
                    xT_r[:, g * 8:(g + 1) * 8, bh * 128:(bh + 1) * 128],
                    stg[:, i % 2, :].rearrange("p (a b) -> p a b", a=8),
                ).then_inc(st1, 16)
            # phase 2: output writes
            for g in range(NGROUP):
                sync.wait_ge(ev2, 4 * (g + 1))
                for bh in range(NBH):
                    sync.dma_start(
                        out[bh * 128:(bh + 1) * 128, g * NG:(g + 1) * NG],
                        so[:, g % 2, bh, :],
                    ).then_inc(sos, 16)

        @block.scalar
        def _(act):
            act.wait_ge(ldw, 16)
            act.activation(wt[:], wt[:], EXP).then_inc(cfA, 1)
            act.wait_ge(cfE, 1)
            for g in range(NGROUP):
                act.wait_ge(gs, 32 * g + 32)          # gb ready
                if g >= 1:
                    act.wait_ge(tt2, 2 * g)           # uf/wf free
                for c in range(NCH):
                    r = g * NCH + c
                    act.activation(                    # u = c3*b + c1
                        uf[:, c * B:(c + 1) * B], gb[:, g % 3, c, :], IDENT,
                        bias=ck[:, 128 + r:129 + r], scale=ck[:, 384 + r:385 + r],
                    ).then_inc(act2, 1)
                    act.activation(                    # w = c2*b + c0
                        wf[:, c * B:(c + 1) * B], gb[:, g % 3, c, :], IDENT,
                        bias=ck[:, r:r + 1], scale=ck[:, 256 + r:257 + r],
                    ).then_inc(act2, 1)

        @block.vector
        def _(vec):
            # coef chain
            vec.wait_ge(cfA, 1)
            e3 = wt.ap().rearrange("p (a b) -> p a b", b=16)
            m3 = mk.ap().rearrange("p (a b) -> p a b", b=16)
            vec.reduce_sum(ssum[:], e3, axis=AX)
            vec.reciprocal(rinv[:], ssum[:])
            vec.wait_ge(ldw, 80)
            for k in range(4):
                o3c = opcf[:, k, :].rearrange("p (a b) -> p a b", b=16)
                vec.tensor_mul(m3, e3, o3c)
                vec.reduce_sum(ckn[:] if k == 0 else ckn[:], m3, axis=AX)
                if k >= 1:
                    vec.wait_ge(cfP, k)
                vec.tensor_mul(ckn[:], ckn[:], rinv[:]).then_inc(cfD, 1)
            vec.wait_ge(cfP, 4)
            vec.tensor_copy(ck[:], pc[:]).then_inc(cfE, 1)
            # phase 1 psum evacs
            for i in range(NBLK):
                for h in range(2):
                    u = 2 * i + h
                    vec.wait_ge(pe1, u + 1)
                    if h == 0 and i >= 2:
                        vec.wait_ge(st1, 16 * (i - 1))
                    vec.tensor_copy(
                        stg[:, i % 2, h * 512:(h + 1) * 512], psum[u % 4][:]
                    ).then_inc(ev1, 1)
            # phase 2 compute + evacs
            for g in range(NGROUP):
                vec.wait_ge(act2, 16 * (g + 1))
                vec.wait_ge(gs, 32 * g + 16)          # ga ready
                vec.tensor_mul(
                    mk[:], ga[:, g % 3].rearrange("p c e -> p (c e)"), uf[:]
                ).then_inc(tt2, 1)
                if g >= 2:
                    vec.wait_ge(pe2, 4 * (g - 1))     # ot slot free
                vec.tensor_add(ot[:, g % 2, :], mk[:], wf[:]).then_inc(tt2, 1)
                for s in range(4):
                    v = 4 * g + s
                    vec.wait_ge(pe2, v + 1)
                    if s == 0 and g >= 2:
                        vec.wait_ge(sos, 32 * (g - 1))
                    bh, sp = s % 2, s // 2
                    vec.tensor_copy(
                        so[:, g % 2, bh, sp * 512:(sp + 1) * 512], psum[v % 4][:]
                    ).then_inc(ev2, 1)

        @block.tensor
        def _(pe):
            pe.wait_ge(ldw, 32)
            for k in range(4):
                pe.wait_ge(cfD, k + 1)
                pe.transpose(pc[:, k * 128:(k + 1) * 128], ckn[:], ident[:]).then_inc(cfP, 1)
            # phase 1
            for i in range(NBLK):
                pe.wait_ge(ld1, 16 * (i + 1))
                for t in range(8):
                    u = 2 * i + t // 4
                    if t % 4 == 0 and u >= 4:
                        pe.wait_ge(ev1, u - 3)
                    inst = pe.transpose(
                        psum[u % 4][:, (t % 4) * 128:(t % 4 + 1) * 128],
                        xin[:, i % 2, t * 128:(t + 1) * 128],
                        ident[:],
                    )
                    if t % 4 == 3:
                        inst.then_inc(pe1, 1)
            # phase 2 back-transposes
            for g in range(NGROUP):
                pe.wait_ge(tt2, 2 * g + 2)
                o3 = ot[:, g % 2, :].rearrange("p (c e) -> p c e", c=NCH)
                for c in range(NCH):
                    for bh in range(2):
                        v = 4 * g + 2 * (c // 4) + bh
                        if c % 4 == 0 and v >= 4:
                            pe.wait_ge(ev2, v - 3)
                        inst = pe.transpose(
                            psum[v % 4][:, (c % 4) * 128:(c % 4 + 1) * 128],
                            o3[:, c, bh * 128:(bh + 1) * 128],
                            ident[:],
                        )
                        if c % 4 == 3:
                            inst.then_inc(pe2, 1)

        @block.gpsimd
        def _(gp):
            gp.wait_ge(st1, 16 * NBLK)
            gp.wait_ge(ldw, 64)
            for g in range(NGROUP):
                if g >= 3:
                    gp.wait_ge(tt2, 2 * (g - 3) + 1)
                gp.dma_gather(
                    ga[:, g % 3], xT[:], iat[:, g * (NG // 16):(g + 1) * (NG // 16)],
                    num_idxs=NG, num_idxs_reg=NG, elem_size=B,
                    single_packet=False, queue_num=(2 * g) % 4,
                ).then_inc(gs, 16)
                if g >= 3:
                    gp.wait_ge(act2, 16 * (g - 2))
                gp.dma_gather(
                    gb[:, g % 3], xT[:], ibt[:, g * (NG // 16):(g + 1) * (NG // 16)],
                    num_idxs=NG, num_idxs_reg=NG, elem_size=B,
                    single_packet=False, queue_num=(2 * g + 1) % 4,
                ).then_inc(gs, 16)

    nc.compile()
    return nc


def wrap_idx(vals):
    """Per-NG-call wrapped int16 tables, concatenated: [128, OUT_DIM//16]."""
    cols = []
    for g in range(NGROUP):
        v = np.asarray(vals[g * NG:(g + 1) * NG])
        arr = v.reshape(NG // 16, 16).T.astype(np.int16)   # [16, NG//16]
        cols.append(np.tile(arr, (8, 1)))                  # [128, NG//16]
    return np.ascontiguousarray(np.concatenate(cols, axis=1))


def kernel(x, idx_a, idx_b, weights, trace=False):
    x = np.asarray(x, dtype=np.float32)
    weights = np.asarray(weights, dtype=np.float32)

    if "nc" not in _cached:
        _cached["nc"] = build_nc()
    nc = _cached["nc"]

    ia_w = wrap_idx(np.asarray(idx_a))
    ib_w = wrap_idx(np.asarray(idx_b))
    wc = np.ascontiguousarray(weights.reshape(128, 128 * 16))
    opc_row = np.repeat(_OP_COEFFS.T[:, None, :], 128, axis=1).reshape(4 * 2048)
    opc = np.ascontiguousarray(np.broadcast_to(opc_row[None, :], (128, 4 * 2048))).astype(np.float32)
    ident = np.eye(128, dtype=np.float32)

    in_maps = [{
        "xs": np.ascontiguousarray(x[k * B:(k + 1) * B]),
        "wc": wc, "opc": opc, "ident": ident, "ia": ia_w, "ib": ib_w,
    } for k in range(NCORES)]
    res = run_bass_kernel_spmd(nc, in_maps, core_ids=list(range(NCORES)), trace=trace)
    out = np.concatenate([r["out"] for r in res.results], axis=0)
    kernel.last_exec_time_ns = res.exec_time_ns
    return out


kernel.last_exec_time_ns = None
